# revision 69
# baseline (speedup 1.0000x reference)
"""HRR self-attention Trainium2 kernel (v2).

Math: reference computes, per head (D=128):
    qkv = x @ W_qkv.T ; q,k,v heads
    kv  = irfft(rfft(k) * rfft(v))          # circular conv bind
    kv  = cumsum(kv, axis=seq)
    out = irfft(rfft(kv) * conj(rfft(q)))   # circular corr unbind
    y   = out @ W_o.T

The rfft/irfft along the head dim are linear maps folded into W_qkv / W_o on
the host: the device computes frequency-domain q,k,v with one GEMM, does the
complex bind / cumsum / unbind elementwise (cumsum commutes with the irfft),
and applies irfft+output projection as a second GEMM.

Sharding: 8 cores = 4 batches x 2 head-groups (4 heads each). Each core emits
a bf16 partial output for its batch; host sums the two head-group partials
per batch in f32.

Frequency packing per head (D=128 -> rfft bins 0..64), lanes per 2-head pair
chunk: re-chunk lane j = Re X[j] (j=0..63 head A; 64..127 head B);
im-chunk lane j: j=0 -> X[64] (Nyquist, real), j>=1 -> Im X[j].
DC/Nyquist lanes {0,64} are real-only bins where the generic complex product
is wrong. Instead of per-lane fixups, masked PSUM->SBUF variants (DVE
tensor_scalar with a per-partition mask, 252ns each on HW) make the generic
6-op complex multiply correct on ALL lanes:
  Kim_m  = Kim * m         (m = 0 on lanes {0,64}, 1 elsewhere)
  Kim_s  = Kim * (1-m)
  Kre_sw = Kre*m + Kim_s
then bind: KVre = Kre*Vre - Kim_m*Vim ; KVim = Kre_sw*Vim + Kim_m*Vre
gives DC*DC in KVre and Nyq*Nyq in KVim at the special lanes. Same trick for
Q at unbind. The cumsum scans run with fp32 internal state but bf16 outputs,
so every bind/unbind multiply is an all-bf16 DVE op (2x rate).

PE schedule (per s-chunk, 128 matmuls of 512 moving cols): the 12 qkv channel
chunks run in three mi-major phases of 4 PSUM banks (A = K0,V0; B = K1,V1;
C = Q0,Q1), with the previous s-chunk's output projection emitted as 2-mo
quarters between phases so every phase transition has PE work while the
PSUM banks turn over through the copy queues. The last s-chunk hoists Q0
before phase B and runs the final out-projection ci-major across all 8 PSUM
banks so the drain only waits on pair-1's short unbind chain. ~60 tiny
warm-up matmuls during the initial DMA wait hold the PE at full clock.
Inputs stream as a few large DMAs in exact consumption order (wf lives in
one wide mi-major SBUF tile); outputs leave as 256KB 2-mo blocks.
"""

import os
import numpy as np
import ml_dtypes

KVAR = os.environ.get("KVAR", "full")

B, S, M, H = 4, 2048, 1024, 8
D = M // H          # 128
NB = D // 2         # 64 bins per half
SC = 512            # sequence chunk
NSC = S // SC       # 4
NMI = M // 128      # 8 contraction chunks
NCO = 4             # U channel chunks per core

BF16 = ml_dtypes.bfloat16


# ---------------------------------------------------------------------------
# Host-side weight fusion
# ---------------------------------------------------------------------------

def _head_blocks(Wh, F):
    """Wh (D, M) spatial head weights -> (re_block, im_block) each (64, M)."""
    FW = F @ Wh  # (65, M) complex
    re = FW.real[0:NB]
    im = np.concatenate([FW.real[NB:NB + 1], FW.imag[1:NB]], axis=0)
    return re, im


def build_tables(W_qkv, W_o):
    """Per-core (WfT [1024,1536] bf16, WoG [512,1024] bf16).

    wf column order: phase A chunks [Kre0,Kim0,Vre0,Vim0,Kre1,Kim1] then
    phase B chunks [Qre0,Qim0,Vre1,Vim1,Qre1,Qim1], 128 cols each.
    """
    W_qkv = np.asarray(W_qkv, dtype=np.float64)
    W_o = np.asarray(W_o, dtype=np.float64)
    F = np.fft.rfft(np.eye(D), axis=-1).T  # (65, 128)
    Wq = W_qkv[0 * M:1 * M].reshape(H, D, M)
    Wk = W_qkv[1 * M:2 * M].reshape(H, D, M)
    Wv = W_qkv[2 * M:3 * M].reshape(H, D, M)

    # irfft basis in packed-channel order [re 0..63, nyq, im 1..63]
    n = np.arange(D)
    f = np.arange(NB)
    Gr = np.cos(2 * np.pi * np.outer(n, f) / D) / D
    Gr[:, 1:] *= 2.0
    Gnyq = np.cos(np.pi * n)[:, None] / D
    Gi = -2.0 * np.sin(2 * np.pi * np.outer(n, f) / D) / D
    G = np.concatenate([Gr, Gnyq, Gi[:, 1:]], axis=1)  # (128, 128)

    tables = []
    for core in range(8):
        g = core % 2
        heads = [4 * g + i for i in range(4)]
        blk = {}
        out_rows = []
        for pair in range(2):
            hA, hB = heads[2 * pair], heads[2 * pair + 1]
            for nm, Wx in (("K", Wk), ("V", Wv), ("Q", Wq)):
                reA, imA = _head_blocks(Wx[hA], F)
                reB, imB = _head_blocks(Wx[hB], F)
                blk[(nm, "re", pair)] = np.concatenate([reA, reB], axis=0)
                blk[(nm, "im", pair)] = np.concatenate([imA, imB], axis=0)
            WoGA = W_o[:, D * hA:D * (hA + 1)] @ G  # (1024, 128)
            WoGB = W_o[:, D * hB:D * (hB + 1)] @ G
            out_rows.append(np.concatenate([WoGA.T[:NB], WoGB.T[:NB]], axis=0))
            out_rows.append(np.concatenate([WoGA.T[NB:], WoGB.T[NB:]], axis=0))
        order = [("K", "re", 0), ("K", "im", 0), ("V", "re", 0), ("V", "im", 0),
                 ("K", "re", 1), ("K", "im", 1), ("V", "re", 1), ("V", "im", 1),
                 ("Q", "re", 0), ("Q", "im", 0), ("Q", "re", 1), ("Q", "im", 1)]
        WfT = np.concatenate([blk[k] for k in order], axis=0).T  # (1024, 1536)
        WoG = np.concatenate(out_rows, axis=0)  # (512, 1024)
        tables.append((np.ascontiguousarray(WfT, dtype=np.float32).astype(BF16),
                       np.ascontiguousarray(WoG, dtype=np.float32).astype(BF16)))
    return tables





# ---------------------------------------------------------------------------
# Device kernel
# ---------------------------------------------------------------------------

def build_kernel(tc, x0d, xr, wf, wo, out, reps=1, loop_iters=None, salt=""):
    import concourse.mybir as mybir
    from contextlib import ExitStack

    nc = tc.nc
    bf16 = mybir.dt.bfloat16
    f32 = mybir.dt.float32
    MULT = mybir.AluOpType.mult
    ADD = mybir.AluOpType.add
    COPY = mybir.ActivationFunctionType.Copy

    with ExitStack() as ctx:
        consts = ctx.enter_context(tc.tile_pool(name="consts", bufs=1))
        xpool = ctx.enter_context(tc.tile_pool(name="xpool", bufs=1))
        wpool = ctx.enter_context(tc.tile_pool(name="wpool", bufs=1))
        qkvp = ctx.enter_context(tc.tile_pool(name="qkvp", bufs=2))
        kvp = ctx.enter_context(tc.tile_pool(name="kvp", bufs=2))
        scanp = ctx.enter_context(tc.tile_pool(name="scanp", bufs=2))
        up = ctx.enter_context(tc.tile_pool(name="up", bufs=2))
        outp = ctx.enter_context(tc.tile_pool(name="outp", bufs=4))
        psq = ctx.enter_context(tc.tile_pool(name="psq", bufs=1, space="PSUM"))
        psop = ctx.enter_context(tc.tile_pool(name="psop", bufs=4, space="PSUM"))

        ones = consts.tile([128, SC], bf16, name=f"ones{salt}")
        nc.vector.memset(ones[:], 1.0)
        warm = consts.tile([128, 64], bf16, name=f"warm{salt}")
        nc.vector.memset(warm[:], 0.0)
        if KVAR != "full":
            # constant stand-in U chunks for stripped-down timing variants
            uconst = [consts.tile([128, SC], bf16, tag=f"uc{i}", name=f"uc{i}")
                      for i in range(4)]
            for t in uconst:
                nc.vector.memset(t[:], 0.001)
        # per-partition masks: m = 1 except lanes {0,64} -> 0 ; ms = 1-m
        m_ap = consts.tile([128, 1], f32, name=f"mpos{salt}")
        ms_ap = consts.tile([128, 1], f32, name=f"mneg{salt}")
        nc.vector.memset(m_ap[:], 1.0)
        nc.vector.memset(m_ap[0:1, :], 0.0)
        nc.vector.memset(m_ap[64:65, :], 0.0)
        nc.vector.memset(ms_ap[:], 0.0)
        nc.vector.memset(ms_ap[0:1, :], 1.0)
        nc.vector.memset(ms_ap[64:65, :], 1.0)

        # ---- input DMA, in consumption order ----
        # wf lives in one wide SBUF tile, column layout [A: mi(8)x4x128 |
        # BC: mi(8)x8x128] matching the host DRAM layout, so the weight
        # stream loads as a few large DMAs paced with consumption:
        # wfA half 1, x0 per-mi 0-3, wfA half 2, x0 4-7, wfBC quarters,
        # then one merged 1MB DMA each for x sc1, wo, x sc2, x sc3.
        wfall = wpool.tile([128, 12288], bf16, tag="wf", name="wfall")
        x0_t = [None] * NMI
        for q in range(4):
            nc.sync.dma_start(out=wfall[:, q * 1024:(q + 1) * 1024],
                              in_=wf[:, q * 1024:(q + 1) * 1024])
            for mi in (2 * q, 2 * q + 1):
                tx = xpool.tile([128, SC], bf16, tag=f"x0_{mi}", name=f"x0_{mi}")
                nc.sync.dma_start(out=tx[:], in_=x0d[mi * 128:(mi + 1) * 128, :])
                x0_t[mi] = tx
        for q in range(4):
            c0 = 4096 + q * 2048
            nc.sync.dma_start(out=wfall[:, c0:c0 + 2048], in_=wf[:, c0:c0 + 2048])
        xr_t = [None] * NSC
        for sc in range(1, NSC):
            t = xpool.tile([128, NMI * SC], bf16, tag=f"xr{sc}", name=f"xr{sc}")
            nc.sync.dma_start(out=t[:], in_=xr[(sc - 1) * 128:sc * 128, :])
            xr_t[sc] = t
            if sc == 1:
                wo_all = wpool.tile([128, 4096], bf16, tag="wo", name="wo_all")
                nc.sync.dma_start(out=wo_all[:], in_=wo[:])

        def x_op(sc, mi):
            if sc == 0:
                return x0_t[mi][:]
            return xr_t[sc][:, mi * SC:(mi + 1) * SC]

        def wf_op(mi, cc):
            if cc < 4:
                c0 = mi * 512 + cc * 128
            else:
                c0 = 4096 + mi * 1024 + (cc - 4) * 128
            return wfall[:, c0:c0 + 128]

        def wo_op(ci, mo):
            return wo_all[:, ci * 1024 + mo * 128:ci * 1024 + (mo + 1) * 128]

        # p-state pre-warm: ~60 tiny matmuls keep the PE continuously busy
        # during the initial DMA wait so the real matmuls start at full clock
        # (the tensor engine drops to half speed for the first 3us after an
        # idle period).
        wps = psop.tile([64, 64], f32, tag="pso", padded_shape=[128, SC],
                        name="warmps")
        for i in range(60):
            nc.tensor.matmul(wps[:], warm[:, 0:64], warm[:, 0:64],
                             start=True, stop=True)

        if loop_iters is not None:
            loop_cm = tc.For_i(
                0, loop_iters, 1,
                hint_engines=(mybir.EngineType.PE, mybir.EngineType.DVE,
                              mybir.EngineType.Activation, mybir.EngineType.Pool,
                              mybir.EngineType.SP))
            loop_cm.__enter__()

        def emit_phase(sc, ccs, tags, pool=None):
            """mi-major accumulation chains -> len(ccs) PSUM tiles."""
            ps = []
            for cc, tg in zip(ccs, tags):
                if pool is None:
                    ps.append(psq.tile([128, SC], f32, tag=f"psq{tg}",
                                       name=f"psq{sc}_{cc}"))
                else:
                    ps.append(pool.tile([128, SC], f32, tag="pso",
                                        name=f"psb{sc}_{cc}"))
            for mi in range(NMI):
                for i, cc in enumerate(ccs):
                    nc.tensor.matmul(
                        ps[i][:], wf_op(mi, cc),
                        x_op(sc, mi), start=(mi == 0), stop=(mi == NMI - 1))
            return ps

        def copy_kv(sc, pair, ps_re, ps_im, pfx):
            """PSUM->SBUF copies for a K or Q chunk pair. Plain re on Act;
            the masked variants are DVE tensor_scalar ops straight from PSUM
            (252ns each on HW, cheapest copy op available)."""
            re = qkvp.tile([128, SC], bf16, tag=f"{pfx}re{pair}",
                           name=f"{pfx}re{sc}_{pair}")
            im_m = qkvp.tile([128, SC], bf16, tag=f"{pfx}imm{pair}",
                             name=f"{pfx}imm{sc}_{pair}")
            im_s = qkvp.tile([128, SC], bf16, tag=f"{pfx}ims{pair}",
                             name=f"{pfx}ims{sc}_{pair}")
            tmp = qkvp.tile([128, SC], bf16, tag=f"{pfx}tmp{pair}",
                            name=f"{pfx}tmp{sc}_{pair}")
            re_sw = qkvp.tile([128, SC], bf16, tag=f"{pfx}resw{pair}",
                              name=f"{pfx}resw{sc}_{pair}")
            nc.scalar.activation(re[:], ps_re[:], COPY)
            nc.vector.tensor_scalar_mul(im_m[:], ps_im[:], m_ap[:])
            nc.vector.tensor_scalar_mul(im_s[:], ps_im[:], ms_ap[:])
            nc.vector.tensor_scalar_mul(tmp[:], ps_re[:], m_ap[:])
            nc.vector.tensor_add(re_sw[:], tmp[:], im_s[:])
            return re, im_m, re_sw

        def copy_plain(sc, pair, ps_re, ps_im, pfx):
            re = qkvp.tile([128, SC], bf16, tag=f"{pfx}re{pair}",
                           name=f"{pfx}re{sc}_{pair}")
            im = qkvp.tile([128, SC], bf16, tag=f"{pfx}im{pair}",
                           name=f"{pfx}im{sc}_{pair}")
            nc.scalar.activation(re[:], ps_re[:], COPY)
            nc.scalar.activation(im[:], ps_im[:], COPY)
            return re, im

        def emit_bind(sc, pair, K, V, cross_eng="gpsimd"):
            xeng = getattr(nc, cross_eng)
            Kre, Kim_m, Kre_sw = K
            Vre, Vim = V
            t1 = kvp.tile([128, SC], bf16, tag=f"t1_{pair}", name=f"t1{sc}_{pair}")
            t2 = kvp.tile([128, SC], bf16, tag=f"t2_{pair}", name=f"t2{sc}_{pair}")
            t3 = kvp.tile([128, SC], bf16, tag=f"t3_{pair}", name=f"t3{sc}_{pair}")
            t4 = kvp.tile([128, SC], bf16, tag=f"t4_{pair}", name=f"t4{sc}_{pair}")
            KVre = kvp.tile([128, SC], bf16, tag=f"kvre{pair}",
                            name=f"kvre{sc}_{pair}")
            KVim = kvp.tile([128, SC], bf16, tag=f"kvim{pair}",
                            name=f"kvim{sc}_{pair}")
            nc.vector.tensor_mul(t1[:], Kre[:], Vre[:])
            nc.vector.tensor_mul(t2[:], Kim_m[:], Vim[:])
            xeng.tensor_mul(t3[:], Kre_sw[:], Vim[:])
            xeng.tensor_mul(t4[:], Kim_m[:], Vre[:])
            nc.vector.tensor_sub(KVre[:], t1[:], t2[:])
            nc.vector.tensor_add(KVim[:], t3[:], t4[:])
            return KVre, KVim

        def emit_scan(sc, pair, KVre, KVim, prev_scan):
            # scan state is fp32 internally regardless of operand dtype; a
            # bf16 output only rounds the stored values (and the tiny carry
            # read at chunk boundaries), but makes every downstream unbind
            # multiply an all-bf16 DVE op (254ns vs 728ns on HW).
            KVre_c = scanp.tile([128, SC], bf16, tag=f"scre{pair}",
                                name=f"scre{sc}_{pair}")
            KVim_c = scanp.tile([128, SC], bf16, tag=f"scim{pair}",
                                name=f"scim{sc}_{pair}")
            init_re = 0.0 if sc == 0 else prev_scan[(pair, 0)][:, SC - 1:SC]
            init_im = 0.0 if sc == 0 else prev_scan[(pair, 1)][:, SC - 1:SC]
            nc.vector.tensor_tensor_scan(
                KVre_c[:], ones[:], KVre[:], init_re, MULT, ADD)
            nc.vector.tensor_tensor_scan(
                KVim_c[:], ones[:], KVim[:], init_im, MULT, ADD)
            prev_scan[(pair, 0)] = KVre_c
            prev_scan[(pair, 1)] = KVim_c
            return KVre_c, KVim_c

        def emit_unbind(sc, pair, C, Q, cross_eng="gpsimd"):
            if KVAR == "nounbind":
                return uconst[0], uconst[1]
            xeng = getattr(nc, cross_eng)
            Cre, Cim = C
            Qre, Qim_m, Qre_sw = Q
            u1 = up.tile([128, SC], bf16, tag=f"u1_{pair}", name=f"u1{sc}_{pair}")
            u2 = up.tile([128, SC], bf16, tag=f"u2_{pair}", name=f"u2{sc}_{pair}")
            u3 = up.tile([128, SC], bf16, tag=f"u3_{pair}", name=f"u3{sc}_{pair}")
            u4 = up.tile([128, SC], bf16, tag=f"u4_{pair}", name=f"u4{sc}_{pair}")
            Ure = up.tile([128, SC], bf16, tag=f"ure{pair}", name=f"ure{sc}_{pair}")
            Uim = up.tile([128, SC], bf16, tag=f"uim{pair}", name=f"uim{sc}_{pair}")
            nc.vector.tensor_mul(u1[:], Cre[:], Qre[:])
            nc.vector.tensor_mul(u2[:], Cim[:], Qim_m[:])
            xeng.tensor_mul(u3[:], Cim[:], Qre_sw[:])
            xeng.tensor_mul(u4[:], Cre[:], Qim_m[:])
            nc.vector.tensor_add(Ure[:], u1[:], u2[:])
            nc.vector.tensor_sub(Uim[:], u3[:], u4[:])
            return Ure, Uim

        def emit_out_tail(sc, U):
            """Last s-chunk: ci-major over all 8 mo for ci 0-1 (only pair-0's
            U needed), borrowing the 4 idle psq banks; then per-mo ci 2-3 +
            stop so the copies/DMAs pipeline under the remaining matmuls."""
            pos = []
            for mo in range(8):
                if mo < 4:
                    pos.append(psop.tile([128, SC], f32, tag="pso",
                                         name=f"pso{sc}_{mo}"))
                else:
                    tg = [2, 3, 0, 1][mo - 4]
                    pos.append(psq.tile([128, SC], f32, tag=f"psq{tg}",
                                        name=f"pso{sc}_{mo}"))
            for ci in (0, 1):
                for mo in range(8):
                    nc.tensor.matmul(
                        pos[mo][:], wo_op(ci, mo), U[ci][:],
                        start=(ci == 0), stop=False)
            sos = [outp.tile([128, 2 * SC], bf16, tag="sot", bufs=4,
                             name=f"sot{sc}_{k}") for k in range(4)]
            for mo in range(8):
                nc.tensor.matmul(pos[mo][:], wo_op(2, mo), U[2][:],
                                 start=False, stop=False)
                nc.tensor.matmul(pos[mo][:], wo_op(3, mo), U[3][:],
                                 start=False, stop=True)
                k, half = divmod(mo, 2)
                dstap = sos[k][:, half * SC:(half + 1) * SC]
                if mo % 2 == 0:
                    nc.scalar.activation(dstap, pos[mo][:], COPY)
                else:
                    nc.vector.tensor_scalar_mul(dstap, pos[mo][:], 1.0)
                r0 = (sc * 4 + k) * 128
                nc.sync.dma_start(
                    out=out[r0:r0 + 128, half * SC:(half + 1) * SC],
                    in_=dstap)

        def emit_out(sc, U, mos):
            """One 2-mo pair: two 4-deep GEMM chains, two half-copies into a
            [128,1024] tile, one 256KB DMA."""
            assert len(mos) == 2 and mos.start % 2 == 0
            k = mos.start // 2
            so = outp.tile([128, 2 * SC], bf16, tag="so",
                           name=f"so{sc}_{k}")
            for half, mo in enumerate(mos):
                po = psop.tile([128, SC], f32, tag="pso",
                               name=f"pso{sc}_{mo}")
                for ci in range(NCO):
                    nc.tensor.matmul(
                        po[:], wo_op(ci, mo), U[ci][:],
                        start=(ci == 0), stop=(ci == NCO - 1))
                if half == 0:
                    nc.scalar.activation(so[:, 0:SC], po[:], COPY)
                else:
                    nc.vector.tensor_scalar_mul(so[:, SC:2 * SC], po[:], 1.0)
            r0 = (sc * 4 + k) * 128
            nc.sync.dma_start(out=out[r0:r0 + 128, :], in_=so[:])

        for rep in range(reps):
            prev_scan = {}
            pend = None
            for sc in range(NSC):
                last = sc == NSC - 1
                if KVAR == "nochain":
                    # matmuls + plain bank-freeing copies + out GEMM only
                    for ph in range(3):
                        ps = emit_phase(sc, [4 * ph + i for i in range(4)],
                                        [0, 1, 2, 3])
                        copy_plain(sc, 0, ps[0], ps[1], f"p{ph}a")
                        copy_plain(sc, 1, ps[2], ps[3], f"p{ph}b")
                        if pend is not None:
                            emit_out(pend[0], pend[1],
                                     range(2 * ph, 2 * ph + 2))
                            if ph == 2:
                                emit_out(pend[0], pend[1], range(6, 8))
                    pend = (sc, uconst)
                    continue
                # phase A: Kre0 Kim0 Vre0 Vim0 -> banks 0-3
                psA = emit_phase(sc, [0, 1, 2, 3], [0, 1, 2, 3])
                K0 = copy_kv(sc, 0, psA[0], psA[1], "k")
                V0 = copy_plain(sc, 0, psA[2], psA[3], "v")
                KV0 = emit_bind(sc, 0, K0, V0)
                C0 = emit_scan(sc, 0, *KV0, prev_scan)
                if pend is not None:
                    emit_out(pend[0], pend[1], range(0, 2))
                if last:
                    # pull Q0 forward so unbind0 overlaps phase B
                    psC0 = emit_phase(sc, [8, 9], [0, 1])
                    Q0 = copy_kv(sc, 0, psC0[0], psC0[1], "q")
                    U0 = emit_unbind(sc, 0, C0, Q0)
                    if pend is not None:
                        emit_out(pend[0], pend[1], range(2, 4))
                # phase B: Kre1 Kim1 Vre1 Vim1 -> banks 0-3 (at sc0 the out
                # quarters don't exist yet, so borrow the idle psop banks to
                # avoid waiting on phase A's copies)
                psB = emit_phase(sc, [4, 5, 6, 7], [0, 1, 2, 3],
                                 pool=psop if sc == 0 else None)
                K1 = copy_kv(sc, 1, psB[0], psB[1], "k")
                V1 = copy_plain(sc, 1, psB[2], psB[3], "v")
                KV1 = emit_bind(sc, 1, K1, V1)
                C1 = emit_scan(sc, 1, *KV1, prev_scan)
                if pend is not None:
                    emit_out(pend[0], pend[1],
                             range(4, 6) if last else range(2, 4))
                # phase C: Q chunks
                if last:
                    psC1 = emit_phase(sc, [10, 11], [0, 1])
                    Q1 = copy_kv(sc, 1, psC1[0], psC1[1], "q")
                    U1 = emit_unbind(sc, 1, C1, Q1, cross_eng="vector")
                    if pend is not None:
                        emit_out(pend[0], pend[1], range(6, 8))
                else:
                    psC = emit_phase(sc, [8, 9, 10, 11], [0, 1, 2, 3])
                    Q0 = copy_kv(sc, 0, psC[0], psC[1], "q")
                    Q1 = copy_kv(sc, 1, psC[2], psC[3], "q")
                    U0 = emit_unbind(sc, 0, C0, Q0)
                    U1 = emit_unbind(sc, 1, C1, Q1)
                    if pend is not None:
                        emit_out(pend[0], pend[1], range(4, 6))
                        emit_out(pend[0], pend[1], range(6, 8))
                if KVAR == "nounbind":
                    pend = (sc, uconst)
                else:
                    pend = (sc, [U0[0], U0[1], U1[0], U1[1]])
            emit_out_tail(pend[0], pend[1])
        if loop_iters is not None:
            loop_cm.__exit__(None, None, None)


def build_bass(reps=1, loop_iters=None, salt=""):
    import concourse.bacc as bacc
    import concourse.tile as tile
    import concourse.mybir as mybir

    nc = bacc.Bacc("TRN2", target_bir_lowering=False, debug=False, num_devices=8)
    x0d = nc.dram_tensor("x0d", [NMI * 128, SC], mybir.dt.bfloat16,
                         kind="ExternalInput")
    xr = nc.dram_tensor("xr", [(NSC - 1) * 128, NMI * SC], mybir.dt.bfloat16,
                        kind="ExternalInput")
    wf = nc.dram_tensor("wf", [128, 12288], mybir.dt.bfloat16,
                        kind="ExternalInput")
    wo = nc.dram_tensor("wo", [128, 4096], mybir.dt.bfloat16,
                        kind="ExternalInput")
    out = nc.dram_tensor("out", [NSC * 4 * 128, 2 * SC], mybir.dt.bfloat16,
                         kind="ExternalOutput")
    with tile.TileContext(nc) as tc:
        build_kernel(tc, x0d[:], xr[:], wf[:], wo[:], out[:], reps=reps,
                     loop_iters=loop_iters, salt=salt)
    nc.compile()
    return nc


_NC_CACHE = {}


def _get_nc(reps=1, loop_iters=None, salt=""):
    key = (reps, loop_iters, salt)
    if key not in _NC_CACHE:
        _NC_CACHE[key] = build_bass(reps, loop_iters, salt)
    return _NC_CACHE[key]


def make_in_maps(x, W_qkv, W_o):
    tables = build_tables(W_qkv, W_o)
    x = np.asarray(x, dtype=np.float32)
    in_maps = []
    for core in range(8):
        b = core // 2
        xT = np.ascontiguousarray(x[b].T)                 # (M, S)
        blocks = xT.reshape(NMI, 128, NSC, SC)            # (mi, p, sc, col)
        x0d = np.ascontiguousarray(
            blocks[:, :, 0, :]).reshape(NMI * 128, SC).astype(BF16)
        # sc 1..3: [p, mi, col] per sc -> one [128, NMI*SC] row-block each
        xr = np.ascontiguousarray(
            blocks[:, :, 1:, :].transpose(2, 1, 0, 3)     # (sc, p, mi, col)
        ).reshape((NSC - 1) * 128, NMI * SC).astype(BF16)
        WfT, WoG = tables[core]
        W4 = WfT.reshape(NMI, 128, 12, 128)               # (mi, p, cc, j)
        wfA = np.ascontiguousarray(
            W4[:, :, 0:4, :].transpose(1, 0, 2, 3)).reshape(128, 4096)
        wfBC = np.ascontiguousarray(
            W4[:, :, 4:12, :].transpose(1, 0, 2, 3)).reshape(128, 8192)
        wf_all = np.concatenate([wfA, wfBC], axis=1)      # (128, 12288)
        wo_all = np.ascontiguousarray(
            WoG.reshape(NCO, 128, M).transpose(1, 0, 2)).reshape(128, 4096)
        in_maps.append({"x0d": x0d, "xr": xr, "wf": np.ascontiguousarray(wf_all),
                        "wo": wo_all.astype(BF16)})
    return in_maps


def combine_outputs(results):
    out = np.empty((B, S, M), dtype=np.float32)
    for b in range(B):
        acc = results[2 * b]["out"].astype(np.float32) + \
            results[2 * b + 1]["out"].astype(np.float32)
        # blocks: (sc, k, 128p, 2half, SC) -> [s, m] with m = (2k+half)*128+p
        acc = acc.reshape(NSC, 4, 128, 2, SC).transpose(0, 4, 1, 3, 2)
        out[b] = acc.reshape(S, M)
    return out


def kernel(x, W_qkv, W_o):
    from concourse.bass_utils import run_bass_kernel_spmd
    nc = _get_nc()
    in_maps = make_in_maps(x, W_qkv, W_o)
    res = run_bass_kernel_spmd(nc, in_maps, core_ids=list(range(8)))
    return combine_outputs(res.results)
